# revision 14
# baseline (speedup 1.0000x reference)
"""CMVN kernel for Trainium2 (Bass/Tile), data-parallel over batch on 8 cores.

Problem: x [64,1,120,4096] f32, length [64] int. Per-sample mean/std over the
first length[b] frames (ddof=1), then normalize the entire padded tensor:
out = (x - mean) / (1e-10 + std). Returns (out, length).

Per-core plan (8 samples, each [120 part, 4096 free] in SBUF):
 - 8x bn_stats over 512-wide chunks -> per-chunk (mean, M2) records
 - combine records of fully-valid chunks via chunk-index mask + TTR
 - ragged partial chunk via runtime register slice (values_load + ds)
   masked with iota < (length % 512)
 - normalize with one ScalarE activation(Identity, scale=1/(eps+std),
   bias=-mean*scale) pass, DMA out.
Host only derives tiny per-sample scalars from `length` (chunk counts,
remainders, 1/n) - all per-element math runs on device.
"""

import numpy as np

import concourse.bass as bass
import concourse.tile as tile
from concourse import mybir
from concourse.bass_utils import run_bass_kernel_spmd

# Problem constants (hardcoded per task contract).
B, C, F, T = 64, 1, 120, 4096
NCORES = 8
BPC = B // NCORES  # samples per core
NCHUNK = 8
CHUNK = T // NCHUNK  # 512 == BN_STATS_FMAX
EPS = 1e-10

f32 = mybir.dt.float32
i32 = mybir.dt.int32
Alu = mybir.AluOpType
Act = mybir.ActivationFunctionType


def build_tile_kernel(tc, x, scal, koff, out):
    """Emit the per-core program.

    x     [BPC, F, T] f32   input samples
    scal  [F, BPC, 4] f32   per-sample scalars (host-replicated over F):
                            kfull, rem, 1/n, 1/(n-1)
    koff  [1, BPC]    i32   element offset of the partial chunk (min(kfull,7)*CHUNK)
    out   [BPC, F, T] f32   normalized output
    """
    nc = tc.nc
    with (
        tc.tile_pool(name="singles", bufs=1) as singles,
        tc.tile_pool(name="xp", bufs=4) as xp,
        tc.tile_pool(name="sp", bufs=4) as sp,
        tc.tile_pool(name="pp", bufs=3) as pp,
    ):
        # Per-sample scalars, host-replicated across all F partitions so a
        # single plain HWDGE DMA (one wait sem) loads them.
        sc = singles.tile([F, BPC, 4], f32)
        nc.sync.dma_start(out=sc, in_=scal[:, :, :])
        ko = singles.tile([1, BPC], i32)
        nc.sync.dma_start(out=ko, in_=koff[:, :])

        zcol = singles.tile([F, 1], f32)
        nc.vector.memset(zcol, 0.0)
        ecol = singles.tile([F, 1], f32)
        nc.vector.memset(ecol, float(EPS))

        # iota constants: 0..CHUNK-1 along free (per-element partial mask), and
        # chunk index repeated per (even, odd) bn_stats group.
        i512 = singles.tile([F, CHUNK], i32)
        nc.gpsimd.iota(i512, pattern=[[1, CHUNK]], base=0, channel_multiplier=0)
        i512f = singles.tile([F, CHUNK], f32)
        nc.vector.tensor_copy(i512f, i512)

        i16 = singles.tile([F, NCHUNK, 2], i32)
        nc.gpsimd.iota(i16, pattern=[[1, NCHUNK], [0, 2]], base=0, channel_multiplier=0)
        i16f = singles.tile([F, NCHUNK, 2], f32)
        nc.vector.tensor_copy(i16f, i16)

        def bcast(col, dims):
            # [F,1] column AP -> [F,*dims] via free-step-0 broadcast
            return bass.AP(
                tensor=col.tensor,
                offset=col.offset,
                ap=[col.ap[0]] + [[0, d] for d in dims],
            )

        for b in range(BPC):
            kcol = sc[:, b, 0:1]     # kfull (f32)
            rcol = sc[:, b, 1:2]     # rem (f32)
            incol = sc[:, b, 2:3]    # 1/n
            inm1col = sc[:, b, 3:4]  # 1/(n-1)

            xt = xp.tile([F, T], f32)
            nc.sync.dma_start(out=xt, in_=x[b, :, :])

            # Chunk stats: one DVE pass total. Record layout per chunk:
            # (cnt_e, mean_e, M2_e, cnt_o, mean_o, M2_o), counts == CHUNK/2.
            rec = sp.tile([F, NCHUNK, 6], f32, tag="rec")
            for ci in range(NCHUNK):
                nc.vector.bn_stats(
                    rec[:, ci, :], xt[:, ci * CHUNK : (ci + 1) * CHUNK]
                )
            means = rec[:, :, 1::3]  # [F, 8, 2]
            m2s = rec[:, :, 2::3]    # [F, 8, 2]

            # Per-group sums: S1g = cnt*mean ; S2g = M2 + cnt*mean^2
            s1g = sp.tile([F, NCHUNK, 2], f32, tag="s1g")
            nc.scalar.mul(s1g, means, float(CHUNK // 2))
            mm = sp.tile([F, NCHUNK, 2], f32, tag="mm")
            nc.vector.tensor_mul(mm, means, means)
            s2g = sp.tile([F, NCHUNK, 2], f32, tag="s2g")
            nc.scalar.mul(s2g, mm, float(CHUNK // 2))
            nc.vector.tensor_add(s2g, s2g, m2s)

            # Valid-chunk mask: chunk_idx < kfull (same for even/odd groups).
            cmask = sp.tile([F, NCHUNK, 2], f32, tag="cmask")
            nc.vector.tensor_tensor(
                out=cmask, in0=i16f, in1=bcast(kcol, [NCHUNK, 2]), op=Alu.is_lt
            )

            # Partial chunk: dynamic slice at koff, mask elements < rem.
            rv = nc.values_load(
                ko[0:1, b : b + 1],
                engines=(mybir.EngineType.DVE,),
                min_val=0,
                max_val=(NCHUNK - 1) * CHUNK,
                skip_runtime_bounds_check=True,
            )
            part = xt[:, bass.ds(rv, CHUNK)]
            pm = pp.tile([F, CHUNK], f32, tag="pm")
            nc.vector.tensor_tensor(
                out=pm, in0=i512f, in1=bcast(rcol, [CHUNK]), op=Alu.is_lt
            )
            pmx = pp.tile([F, CHUNK], f32, tag="pmx")
            nc.vector.tensor_mul(pmx, part, pm)
            s1p = sp.tile([F, 1], f32, tag="s1p")
            nc.vector.tensor_reduce(
                out=s1p, in_=pmx, axis=mybir.AxisListType.X, op=Alu.add
            )
            junk = pp.tile([F, CHUNK], f32, tag="junk")
            s2p = sp.tile([F, 1], f32, tag="s2p")
            nc.scalar.activation(
                out=junk, in_=pmx, func=Act.Square, accum_out=s2p
            )

            # Masked totals + partial-chunk sums.
            j16a = sp.tile([F, NCHUNK, 2], f32, tag="j16a")
            nc.vector.tensor_mul(j16a, s1g, cmask)
            s1f = sp.tile([F, 1], f32, tag="s1f")
            nc.vector.tensor_reduce(
                out=s1f, in_=j16a.rearrange("p a b -> p (a b)"),
                axis=mybir.AxisListType.X, op=Alu.add,
            )
            j16b = sp.tile([F, NCHUNK, 2], f32, tag="j16b")
            nc.vector.tensor_mul(j16b, s2g, cmask)
            s2f = sp.tile([F, 1], f32, tag="s2f")
            nc.vector.tensor_reduce(
                out=s2f, in_=j16b.rearrange("p a b -> p (a b)"),
                axis=mybir.AxisListType.X, op=Alu.add,
            )
            s1 = sp.tile([F, 1], f32, tag="s1")
            nc.vector.tensor_add(s1, s1f, s1p)
            s2 = sp.tile([F, 1], f32, tag="s2")
            nc.vector.tensor_add(s2, s2f, s2p)

            # mean = S1/n ; var = max((S2 - S1*mean)/(n-1), 0)
            mean = sp.tile([F, 1], f32, tag="mean")
            nc.vector.tensor_mul(mean, s1, incol)
            t2 = sp.tile([F, 1], f32, tag="t2")
            nc.vector.tensor_mul(t2, s1, mean)
            t3 = sp.tile([F, 1], f32, tag="t3")
            nc.vector.tensor_tensor(out=t3, in0=s2, in1=t2, op=Alu.subtract)
            var = sp.tile([F, 1], f32, tag="var")
            nc.vector.tensor_mul(var, t3, inm1col)
            nc.vector.tensor_tensor(out=var, in0=var, in1=zcol, op=Alu.max)
            std = sp.tile([F, 1], f32, tag="std")
            nc.scalar.sqrt(std, var)
            den = sp.tile([F, 1], f32, tag="den")
            nc.vector.tensor_add(den, std, ecol)
            inv = sp.tile([F, 1], f32, tag="inv")
            nc.vector.reciprocal(inv, den)
            nmi0 = sp.tile([F, 1], f32, tag="nmi0")
            nc.vector.tensor_mul(nmi0, mean, inv)
            nmi = sp.tile([F, 1], f32, tag="nmi")
            nc.vector.tensor_tensor(out=nmi, in0=zcol, in1=nmi0, op=Alu.subtract)

            # out = x*inv - mean*inv on ScalarE, in place; then store.
            nc.scalar.activation(
                out=xt, in_=xt, func=Act.Identity, bias=nmi, scale=inv
            )
            nc.sync.dma_start(out=out[b, :, :], in_=xt)


def _split_multi_waits(nc):
    """This walrus build allows at most one sync wait per instruction.

    Tile emits several; hoist all but one onto sequencer-only
    InstEventSemaphore instructions spliced immediately before the
    instruction on the same engine (order-preserving, so semantics are
    unchanged: waits are a conjunction).
    """
    import copy

    import bass_rust

    # Template event-sem instruction from a scratch Bass.
    scratch = bass.Bass("TRN2")
    with scratch.semaphore("tmpl_sem") as s:
        tmpl = scratch.vector.wait_ge(s, 1).ins

    uid = [0]

    def make_wait(engine, wait):
        ins = copy.copy(tmpl)
        uid[0] += 1
        ins.name = f"WSPLIT-{uid[0]}"
        ins.engine = engine
        ins.sync_info = bass_rust.SyncInfo(on_wait=[wait], on_update=[])
        return ins

    for fn in nc.m.functions:
        for blk in fn.blocks:
            out_list = []
            changed = False
            for ins in blk.instructions:
                si = ins.sync_info
                waits = list(si.on_wait) if (si and si.on_wait) else []
                if len(waits) > 1:
                    changed = True
                    for w in waits[:-1]:
                        out_list.append(make_wait(ins.engine, w))
                    ins.sync_info = bass_rust.SyncInfo(
                        on_wait=[waits[-1]], on_update=list(si.on_update or [])
                    )
                out_list.append(ins)
            if changed:
                blk.instructions = out_list


def _replace_range_clear(nc):
    """This walrus rejects EVENT_SEMAPHORE_RANGE_CLEAR ("ISA wrong length").

    The clear runs after a full barrier in a loop-free program, so each
    semaphore's value there is statically known: the sum of all updates
    from instructions before the clear.  Replace the range-clear with
    explicit sem-dec event-semaphore instructions restoring each sem in
    the range to zero.
    """
    import copy

    import bass_rust

    scratch = bass.Bass("TRN2")
    with scratch.semaphore("tmpl_sem") as s:
        tmpl = scratch.vector.wait_ge(s, 1).ins

    for fn in nc.m.functions:
        # Pass 1: locate clears and accumulate pre-clear totals.
        clears = []  # (blk, idx, ins)
        totals = {}
        names = {}
        seen_clear = False
        for blk in fn.blocks:
            for idx, ins in enumerate(blk.instructions):
                if (
                    type(ins).__name__ == "InstISA"
                    and getattr(ins, "isa_opcode", None) == 176
                ):
                    clears.append((blk, idx, ins))
                    seen_clear = True
                    continue
                si = ins.sync_info
                if not seen_clear and si and si.on_update:
                    for u in si.on_update:
                        sign = -1 if u.update_mode in ("sem-dec", "sem-sub-imm") else 1
                        totals[u.id] = totals.get(u.id, 0) + sign * u.update_value
                        names[u.id] = u.ant_name
        assert len(clears) <= 1, "multiple sem range clears unsupported"
        for blk, idx, ins in clears:
            lo = ins.ant_dict["range_first"]
            hi = ins.ant_dict["range_last"]
            repl = []
            uid = 0
            for sid in range(lo, hi + 1):
                v = totals.get(sid, 0)
                assert v >= 0, f"sem {sid} negative at clear: {v}"
                if v == 0:
                    continue
                dec = copy.copy(tmpl)
                uid += 1
                dec.name = f"SEMCLR-{ins.name}-{uid}"
                dec.engine = ins.engine
                dec.sync_info = bass_rust.SyncInfo(
                    on_wait=[],
                    on_update=[
                        bass_rust.SyncUpdate(
                            sync_type="semaphore",
                            id=sid,
                            ant_name=names.get(sid, f"sem{sid}"),
                            update_mode="sem-sub-imm",
                            update_value=v,
                            update_reg=None,
                        )
                    ],
                )
                repl.append(dec)
            blk.instructions = (
                blk.instructions[:idx] + repl + blk.instructions[idx + 1 :]
            )


_NC_CACHE = None


def _get_nc():
    global _NC_CACHE
    if _NC_CACHE is None:
        nc = bass.Bass("TRN2")
        x = nc.dram_tensor("x", [BPC, F, T], f32, kind="ExternalInput")
        scal = nc.dram_tensor("scal", [F, BPC, 4], f32, kind="ExternalInput")
        koff = nc.dram_tensor("koff", [1, BPC], i32, kind="ExternalInput")
        out = nc.dram_tensor("out", [BPC, F, T], f32, kind="ExternalOutput")
        with tile.TileContext(nc) as tc:
            build_tile_kernel(tc, x, scal, koff, out)
        _split_multi_waits(nc)
        _replace_range_clear(nc)
        _NC_CACHE = nc
    return _NC_CACHE


def host_scalars(length):
    """Derive per-sample scalar inputs from length on host."""
    L = np.asarray(length).astype(np.int64)
    kfull = L // CHUNK
    rem = L % CHUNK
    koff = (np.minimum(kfull, NCHUNK - 1) * CHUNK).astype(np.int32)
    n = L.astype(np.float64)
    scal = np.stack(
        [
            kfull.astype(np.float64),
            rem.astype(np.float64),
            1.0 / n,
            1.0 / (n - 1.0),
        ],
        axis=1,
    ).astype(np.float32)
    return scal, koff


TRACE = False
LAST_RESULTS = None


def kernel(x, length):
    global LAST_RESULTS
    x_np = np.asarray(x)
    length_np = np.asarray(length)
    assert x_np.shape == (B, C, F, T), x_np.shape
    x_np = np.ascontiguousarray(x_np.reshape(B, F, T).astype(np.float32, copy=False))

    scal, koff = host_scalars(length_np)

    in_maps = []
    for c in range(NCORES):
        sl = slice(c * BPC, (c + 1) * BPC)
        in_maps.append(
            {
                "x": x_np[sl],
                "scal": np.ascontiguousarray(
                    np.broadcast_to(scal[sl][None, :, :], (F, BPC, 4))
                ),
                "koff": np.ascontiguousarray(koff[sl].reshape(1, BPC)),
            }
        )

    nc = _get_nc()
    res = run_bass_kernel_spmd(nc, in_maps, core_ids=list(range(NCORES)), trace=TRACE)
    LAST_RESULTS = res

    out = np.empty((B, F, T), dtype=np.float32)
    for c in range(NCORES):
        out[c * BPC : (c + 1) * BPC] = res.results[c]["out"]
    return out.reshape(B, C, F, T), length_np


# revision 39
# speedup vs baseline: 57025.6265x; 57025.6265x over previous
"""CMVN kernel for Trainium2 (Bass/Tile), data-parallel over batch on 8 cores.

Problem: x [64,1,120,4096] f32, length [64] int. Per-sample mean/std over the
first length[b] frames (ddof=1), then normalize the entire padded tensor:
out = (x - mean) / (1e-10 + std). Returns (out, length).

Per-core plan (8 samples, each [120 part, 4096 free] in SBUF):
 - 8x bn_stats over 512-wide chunks -> per-chunk (mean, M2) records
 - ragged partial chunk: runtime register slice (values_load + ds) masked
   by (iota < length % 512) on GpSimd, multiplied on DVE, then bn_stats on
   the masked window forms a 9th record (zeros-as-data algebra is exact
   for plain S1/S2 sums)
 - record algebra / chunk masking / stats finalize batched per PAIR of
   samples via strided + step-0 broadcast APs (cuts tiny-op overhead)
 - normalize with one ScalarE activation(Identity, scale=1/(eps+std),
   bias=-mean*scale) pass per sample, DMA out.
Host only derives tiny per-sample scalars from `length` (chunk counts,
remainders, 1/n) - all per-element math runs on device.
"""

import numpy as np

import concourse.bass as bass
import concourse.tile as tile
from concourse import mybir
from concourse.bass_utils import run_bass_kernel_spmd

# Problem constants (hardcoded per task contract).
B, C, F, T = 64, 1, 120, 4096
NCORES = 8
BPC = B // NCORES  # samples per core
NCHUNK = 8
CHUNK = T // NCHUNK  # 512 == BN_STATS_FMAX
NREC = NCHUNK + 1  # 8 full chunks + 1 masked partial window
GRP = 2  # samples per algebra group
SCW = 5  # scal table width (kfull, rem, 1/n, 1/(n-1), koff-as-f32-bits)
EPS = 1e-10

f32 = mybir.dt.float32
i32 = mybir.dt.int32
Alu = mybir.AluOpType
Act = mybir.ActivationFunctionType


def _ap(col, dims_steps):
    """AP over `col`'s tensor starting at col's offset with explicit
    [step, count] free dims (partition dim copied from col)."""
    return bass.AP(
        tensor=col.tensor,
        offset=col.offset,
        ap=[col.ap[0]] + [[s, n] for s, n in dims_steps],
    )


def build_tile_kernel(tc, x, scal, out, reps=1, static_partial=False):
    """Emit the per-core program.

    x     [BPC, F, T] f32   input samples
    scal  [F, BPC, 5] f32   per-sample scalars (host-replicated over F):
                            kfull, rem, 1/n, 1/(n-1), bitcast-int32 element
                            offset of the partial chunk (min(kfull,7)*CHUNK)
    out   [BPC, F, T] f32   normalized output
    reps  repeat the whole pipeline (timing harness only; reps=1 for real use)
    """
    nc = tc.nc
    with (
        tc.tile_pool(name="singles", bufs=1) as singles,
        tc.tile_pool(name="xp", bufs=8) as xp,
        tc.tile_pool(name="sp", bufs=3) as sp,
        tc.tile_pool(name="pp", bufs=3) as pp,
    ):
        # First sample load leads (nothing depends on it and it's on the
        # critical DMA path); the tiny scalar table follows immediately and
        # lands long before the first pair's algebra needs it.
        xt0 = xp.tile([F, T], f32, tag="xt")
        nc.sync.dma_start(out=xt0, in_=x[0, :, :])

        sc = singles.tile([F, BPC, 5], f32)
        nc.sync.dma_start(out=sc, in_=scal[:, :, :])

        rvs = [
            nc.values_load(
                sc[0:1, b, 4:5].bitcast(i32),
                engines=(mybir.EngineType.DVE,),
                min_val=0,
                max_val=(NCHUNK - 1) * CHUNK,
                skip_runtime_bounds_check=True,
            )
            for b in range(BPC)
        ]

        zcol = singles.tile([F, 1], f32)
        nc.vector.memset(zcol, 0.0)
        ecol = singles.tile([F, 1], f32)
        nc.vector.memset(ecol, float(EPS))

        # iota constants: 0..CHUNK-1 (partial-window mask) and the record
        # chunk-index table (chunk c for records 0-7, -1 for the always-on
        # partial record), repeated per bn_stats even/odd group.
        i512 = singles.tile([F, CHUNK], i32)
        nc.gpsimd.iota(i512, pattern=[[1, CHUNK]], base=0, channel_multiplier=0)
        i512f = singles.tile([F, CHUNK], f32)
        nc.vector.tensor_copy(i512f, i512)

        i18 = singles.tile([F, NREC, 2], i32)
        nc.gpsimd.iota(i18, pattern=[[1, NREC], [0, 2]], base=0, channel_multiplier=0)
        i18f = singles.tile([F, NREC, 2], f32)
        nc.vector.tensor_copy(i18f, i18)
        nc.vector.memset(i18f[:, NCHUNK, :], -1.0)

        # body emitted `reps` times for the timing harness; reps=1 in production
        xtiles = []
        for rep, g in [(r, gg) for r in range(reps) for gg in range(BPC // GRP)]:
            if g == 0:
                # per-rep: kick off all sample loads back-to-back
                xtiles = []
                for b in range(BPC):
                    if rep == 0 and b == 0:
                        xtiles.append(xt0)
                        continue
                    xt = xp.tile([F, T], f32, tag="xt")
                    nc.sync.dma_start(out=xt, in_=x[b, :, :])
                    xtiles.append(xt)
            b0 = g * GRP
            # Pair-strided scalar APs ([F, GRP] views into sc).
            kpair = sc[:, b0 : b0 + GRP, 0]
            rcol = sc[:, b0 : b0 + GRP, 1:2]  # base for broadcast
            inpair = sc[:, b0 : b0 + GRP, 2]
            inm1pair = sc[:, b0 : b0 + GRP, 3]

            xts = []
            grec = sp.tile([F, GRP, NREC, 6], f32, tag="grec")

            # Partial-window masks for the whole pair: pm[i, t] = (t < rem_i).
            pmslab = pp.tile([F, GRP, CHUNK], f32, tag="pmslab")
            nc.vector.tensor_tensor(
                out=pmslab,
                in0=_ap(i512f[:, 0:1], [(0, GRP), (1, CHUNK)]),
                in1=_ap(rcol, [(SCW, GRP), (0, CHUNK)]),
                op=Alu.is_lt,
            )

            for i in range(GRP):
                b = b0 + i
                xt = xtiles[b]
                xts.append(xt)

                for ci in range(NCHUNK):
                    nc.vector.bn_stats(
                        grec[:, i, ci, :], xt[:, ci * CHUNK : (ci + 1) * CHUNK]
                    )

                pmx = pp.tile([F, CHUNK], f32, tag="pmx")
                if static_partial:
                    # timing-harness mode: same op shapes, no register APs
                    part = xt[:, (NCHUNK - 1) * CHUNK :]
                else:
                    part = xt[:, bass.ds(rvs[b], CHUNK)]
                nc.vector.tensor_mul(pmx, part, pmslab[:, i, :])
                nc.vector.bn_stats(grec[:, i, NCHUNK, :], pmx)

            # ---- record algebra, batched over the pair ([F, GRP*NREC*2]) ----
            means = grec[:, :, :, 1::3]  # [F, GRP, NREC, 2]
            m2s = grec[:, :, :, 2::3]

            s1g = sp.tile([F, GRP, NREC, 2], f32, tag="s1g")
            nc.scalar.mul(s1g, means, float(CHUNK // 2))
            mm = sp.tile([F, GRP, NREC, 2], f32, tag="mm")
            nc.vector.tensor_mul(mm, means, means)
            s2g = sp.tile([F, GRP, NREC, 2], f32, tag="s2g")
            nc.scalar.mul(s2g, mm, float(CHUNK // 2))
            nc.vector.tensor_add(s2g, s2g, m2s)

            cmask = sp.tile([F, GRP, NREC, 2], f32, tag="cmask")
            nc.vector.tensor_tensor(
                out=cmask,
                in0=_ap(i18f[:, 0:1, 0:1], [(0, GRP), (2, NREC), (1, 2)]),
                in1=_ap(kpair[:, 0:1], [(SCW, GRP), (0, NREC), (0, 2)]),
                op=Alu.is_lt,
            )
            nc.vector.tensor_mul(s1g, s1g, cmask)
            nc.vector.tensor_mul(s2g, s2g, cmask)

            s1 = sp.tile([F, GRP], f32, tag="s1")
            nc.vector.tensor_reduce(
                out=s1, in_=s1g.rearrange("p g a b -> p g (a b)"),
                axis=mybir.AxisListType.X, op=Alu.add,
            )
            s2 = sp.tile([F, GRP], f32, tag="s2")
            nc.vector.tensor_reduce(
                out=s2, in_=s2g.rearrange("p g a b -> p g (a b)"),
                axis=mybir.AxisListType.X, op=Alu.add,
            )

            # ---- finalize, batched over the pair ([F, GRP]) ----
            mean = sp.tile([F, GRP], f32, tag="mean")
            nc.vector.tensor_mul(mean, s1, inpair)
            t2 = sp.tile([F, GRP], f32, tag="t2")
            nc.vector.tensor_mul(t2, s1, mean)
            t3 = sp.tile([F, GRP], f32, tag="t3")
            nc.vector.tensor_tensor(out=t3, in0=s2, in1=t2, op=Alu.subtract)
            var = sp.tile([F, GRP], f32, tag="var")
            nc.vector.tensor_mul(var, t3, inm1pair)
            nc.vector.tensor_tensor(
                out=var, in0=var, in1=_ap(zcol, [(0, GRP)]), op=Alu.max
            )
            std = sp.tile([F, GRP], f32, tag="std")
            nc.scalar.sqrt(std, var)
            den = sp.tile([F, GRP], f32, tag="den")
            nc.vector.tensor_tensor(
                out=den, in0=std, in1=_ap(ecol, [(0, GRP)]), op=Alu.add
            )
            inv = sp.tile([F, GRP], f32, tag="inv")
            nc.vector.reciprocal(inv, den)
            nmi = sp.tile([F, GRP], f32, tag="nmi")
            nc.vector.tensor_mul(nmi, mean, inv)
            nc.vector.tensor_tensor(
                out=nmi, in0=_ap(zcol, [(0, GRP)]), in1=nmi, op=Alu.subtract
            )

            # ---- normalize + store per sample ----
            for i in range(GRP):
                b = b0 + i
                xt = xts[i]
                nc.scalar.activation(
                    out=xt, in_=xt, func=Act.Identity,
                    bias=nmi[:, i : i + 1], scale=inv[:, i : i + 1],
                )
                nc.sync.dma_start(out=out[b, :, :], in_=xt)


def _split_multi_waits(nc):
    """This walrus build allows at most one sync wait per instruction.

    Tile emits several; hoist all but one onto sequencer-only
    InstEventSemaphore instructions spliced immediately before the
    instruction on the same engine (order-preserving, so semantics are
    unchanged: waits are a conjunction).
    """
    import copy

    import bass_rust

    scratch = bass.Bass("TRN2")
    with scratch.semaphore("tmpl_sem") as s:
        tmpl = scratch.vector.wait_ge(s, 1).ins

    uid = [0]

    def make_wait(engine, wait):
        ins = copy.copy(tmpl)
        uid[0] += 1
        ins.name = f"WSPLIT-{uid[0]}"
        ins.engine = engine
        ins.sync_info = bass_rust.SyncInfo(on_wait=[wait], on_update=[])
        return ins

    for fn in nc.m.functions:
        for blk in fn.blocks:
            out_list = []
            changed = False
            for ins in blk.instructions:
                si = ins.sync_info
                waits = list(si.on_wait) if (si and si.on_wait) else []
                if len(waits) > 1:
                    changed = True
                    for w in waits[:-1]:
                        out_list.append(make_wait(ins.engine, w))
                    ins.sync_info = bass_rust.SyncInfo(
                        on_wait=[waits[-1]], on_update=list(si.on_update or [])
                    )
                out_list.append(ins)
            if changed:
                blk.instructions = out_list


def _replace_range_clear(nc):
    """This walrus rejects EVENT_SEMAPHORE_RANGE_CLEAR ("ISA wrong length").

    The clear runs after a full barrier in a loop-free program, so each
    semaphore's value there is statically known: the sum of all updates
    from instructions before the clear.  Replace the range-clear with
    explicit sem-sub-imm event-semaphore instructions restoring each sem
    in the range to zero.
    """
    import copy

    import bass_rust

    scratch = bass.Bass("TRN2")
    with scratch.semaphore("tmpl_sem") as s:
        tmpl = scratch.vector.wait_ge(s, 1).ins

    for fn in nc.m.functions:
        clears = []
        totals = {}
        names = {}
        seen_clear = False
        for blk in fn.blocks:
            for idx, ins in enumerate(blk.instructions):
                if (
                    type(ins).__name__ == "InstISA"
                    and getattr(ins, "isa_opcode", None) == 176
                ):
                    clears.append((blk, idx, ins))
                    seen_clear = True
                    continue
                si = ins.sync_info
                if not seen_clear and si and si.on_update:
                    for u in si.on_update:
                        sign = -1 if u.update_mode in ("sem-dec", "sem-sub-imm") else 1
                        totals[u.id] = totals.get(u.id, 0) + sign * u.update_value
                        names[u.id] = u.ant_name
        assert len(clears) <= 1, "multiple sem range clears unsupported"
        for blk, idx, ins in clears:
            lo = ins.ant_dict["range_first"]
            hi = ins.ant_dict["range_last"]
            repl = []
            uid = 0
            for sid in range(lo, hi + 1):
                v = totals.get(sid, 0)
                assert v >= 0, f"sem {sid} negative at clear: {v}"
                if v == 0:
                    continue
                dec = copy.copy(tmpl)
                uid += 1
                dec.name = f"SEMCLR-{ins.name}-{uid}"
                dec.engine = ins.engine
                dec.sync_info = bass_rust.SyncInfo(
                    on_wait=[],
                    on_update=[
                        bass_rust.SyncUpdate(
                            sync_type="semaphore",
                            id=sid,
                            ant_name=names.get(sid, f"sem{sid}"),
                            update_mode="sem-sub-imm",
                            update_value=v,
                            update_reg=None,
                        )
                    ],
                )
                repl.append(dec)
            blk.instructions = (
                blk.instructions[:idx] + repl + blk.instructions[idx + 1 :]
            )


_NC_CACHE = None


def _get_nc():
    global _NC_CACHE
    if _NC_CACHE is None:
        nc = bass.Bass("TRN2")
        x = nc.dram_tensor("x", [BPC, F, T], f32, kind="ExternalInput")
        scal = nc.dram_tensor("scal", [F, BPC, 5], f32, kind="ExternalInput")
        out = nc.dram_tensor("out", [BPC, F, T], f32, kind="ExternalOutput")
        with tile.TileContext(nc) as tc:
            build_tile_kernel(tc, x, scal, out)
        _split_multi_waits(nc)
        _replace_range_clear(nc)
        _NC_CACHE = nc
    return _NC_CACHE


def host_scalars(length):
    """Derive per-sample scalar inputs from length on host."""
    L = np.asarray(length).astype(np.int64)
    kfull = L // CHUNK
    rem = L % CHUNK
    koff = (np.minimum(kfull, NCHUNK - 1) * CHUNK).astype(np.int32)
    n = L.astype(np.float64)
    scal = np.stack(
        [
            kfull.astype(np.float64),
            rem.astype(np.float64),
            1.0 / n,
            1.0 / (n - 1.0),
        ],
        axis=1,
    ).astype(np.float32)
    # column 4: partial-chunk element offset, int32 bits viewed as f32
    scal = np.concatenate([scal, koff.view(np.float32)[:, None]], axis=1)
    return scal


TRACE = False
LAST_RESULTS = None


def kernel(x, length):
    global LAST_RESULTS
    x_np = np.asarray(x)
    length_np = np.asarray(length)
    assert x_np.shape == (B, C, F, T), x_np.shape
    x_np = np.ascontiguousarray(x_np.reshape(B, F, T).astype(np.float32, copy=False))

    scal = host_scalars(length_np)

    in_maps = []
    for c in range(NCORES):
        sl = slice(c * BPC, (c + 1) * BPC)
        in_maps.append(
            {
                "x": x_np[sl],
                "scal": np.ascontiguousarray(
                    np.broadcast_to(scal[sl][None, :, :], (F, BPC, 5))
                ),
            }
        )

    nc = _get_nc()
    res = run_bass_kernel_spmd(nc, in_maps, core_ids=list(range(NCORES)), trace=TRACE)
    LAST_RESULTS = res

    out = np.empty((B, F, T), dtype=np.float32)
    for c in range(NCORES):
        out[c * BPC : (c + 1) * BPC] = res.results[c]["out"]
    return out.reshape(B, C, F, T), length_np


# revision 43
# speedup vs baseline: 57893.7532x; 1.0152x over previous
"""CMVN kernel for Trainium2 (Bass/Tile), data-parallel over batch on 8 cores.

Problem: x [64,1,120,4096] f32, length [64] int. Per-sample mean/std over the
first length[b] frames (ddof=1), then normalize the entire padded tensor:
out = (x - mean) / (1e-10 + std). Returns (out, length).

Per-core plan (8 samples, each [120 part, 4096 free] in SBUF):
 - 8x bn_stats over 512-wide chunks -> per-chunk (mean, M2) records
 - ragged partial chunk: runtime register slice (values_load + ds) masked
   by (iota < length % 512) on GpSimd, multiplied on DVE, then bn_stats on
   the masked window forms a 9th record (zeros-as-data algebra is exact
   for plain S1/S2 sums)
 - record algebra / chunk masking / stats finalize batched per PAIR of
   samples via strided + step-0 broadcast APs (cuts tiny-op overhead)
 - normalize with one ScalarE activation(Identity, scale=1/(eps+std),
   bias=-mean*scale) pass per sample, DMA out.
Host only derives tiny per-sample scalars from `length` (chunk counts,
remainders, 1/n) - all per-element math runs on device.
"""

import numpy as np

import concourse.bass as bass
import concourse.tile as tile
from concourse import mybir
from concourse.bass_utils import run_bass_kernel_spmd

# Problem constants (hardcoded per task contract).
B, C, F, T = 64, 1, 120, 4096
NCORES = 8
BPC = B // NCORES  # samples per core
NCHUNK = 8
CHUNK = T // NCHUNK  # 512 == BN_STATS_FMAX
NREC = NCHUNK + 1  # 8 full chunks + 1 masked partial window
GRP = 2  # samples per algebra group
SCW = 5  # scal table width (kfull, rem, 1/n, 1/(n-1), koff-as-f32-bits)
EPS = 1e-10

f32 = mybir.dt.float32
i32 = mybir.dt.int32
Alu = mybir.AluOpType
Act = mybir.ActivationFunctionType


def _ap(col, dims_steps):
    """AP over `col`'s tensor starting at col's offset with explicit
    [step, count] free dims (partition dim copied from col)."""
    return bass.AP(
        tensor=col.tensor,
        offset=col.offset,
        ap=[col.ap[0]] + [[s, n] for s, n in dims_steps],
    )


def build_tile_kernel(tc, x, scal, out, reps=1, static_partial=False):
    """Emit the per-core program.

    x     [BPC, F, T] f32   input samples
    scal  [F, BPC, 5] f32   per-sample scalars (host-replicated over F):
                            kfull, rem, 1/n, 1/(n-1), bitcast-int32 element
                            offset of the partial chunk (min(kfull,7)*CHUNK)
    out   [BPC, F, T] f32   normalized output
    reps  repeat the whole pipeline (timing harness only; reps=1 for real use)
    """
    nc = tc.nc
    with (
        tc.tile_pool(name="singles", bufs=1) as singles,
        tc.tile_pool(name="xp", bufs=8) as xp,
        tc.tile_pool(name="sp", bufs=3) as sp,
        tc.tile_pool(name="pp", bufs=3) as pp,
    ):
        # First sample load leads (nothing depends on it and it's on the
        # critical DMA path); the tiny scalar table follows immediately and
        # lands long before the first pair's algebra needs it.
        xt0 = xp.tile([F, T], f32, tag="xt")
        nc.sync.dma_start(out=xt0, in_=x[0, :, :])

        sc = singles.tile([F, BPC, 5], f32)
        nc.sync.dma_start(out=sc, in_=scal[:, :, :])

        rvs = [
            nc.values_load(
                sc[0:1, b, 4:5].bitcast(i32),
                engines=(mybir.EngineType.DVE,),
                min_val=0,
                max_val=(NCHUNK - 1) * CHUNK,
                skip_runtime_bounds_check=True,
            )
            for b in range(BPC)
        ]

        zcol = singles.tile([F, 1], f32)
        nc.vector.memset(zcol, 0.0)
        ecol = singles.tile([F, 1], f32)
        nc.vector.memset(ecol, float(EPS))

        # iota constants: 0..CHUNK-1 (partial-window mask) and the record
        # chunk-index table (chunk c for records 0-7, -1 for the always-on
        # partial record), repeated per bn_stats even/odd group.
        i512 = singles.tile([F, CHUNK], i32)
        nc.gpsimd.iota(i512, pattern=[[1, CHUNK]], base=0, channel_multiplier=0)
        i512f = singles.tile([F, CHUNK], f32)
        nc.vector.tensor_copy(i512f, i512)

        i18 = singles.tile([F, NREC, 2], i32)
        nc.gpsimd.iota(i18, pattern=[[1, NREC], [0, 2]], base=0, channel_multiplier=0)
        i18f = singles.tile([F, NREC, 2], f32)
        nc.vector.tensor_copy(i18f, i18)
        nc.vector.memset(i18f[:, NCHUNK, :], -1.0)

        # body emitted `reps` times for the timing harness; reps=1 in production
        xtiles = []
        for rep, g in [(r, gg) for r in range(reps) for gg in range(BPC // GRP)]:
            if g == 0:
                # per-rep: kick off all sample loads back-to-back
                xtiles = []
                for b in range(BPC):
                    if rep == 0 and b == 0:
                        xtiles.append(xt0)
                        continue
                    xt = xp.tile([F, T], f32, tag="xt")
                    nc.sync.dma_start(out=xt, in_=x[b, :, :])
                    xtiles.append(xt)
            b0 = g * GRP
            # Pair-strided scalar APs ([F, GRP] views into sc).
            kpair = sc[:, b0 : b0 + GRP, 0]
            rcol = sc[:, b0 : b0 + GRP, 1:2]  # base for broadcast
            inpair = sc[:, b0 : b0 + GRP, 2]
            inm1pair = sc[:, b0 : b0 + GRP, 3]

            xts = []
            grec = sp.tile([F, GRP, NREC, 6], f32, tag="grec")

            # Partial-window masks for the whole pair: pm[i, t] = (t < rem_i).
            pmslab = pp.tile([F, GRP, CHUNK], f32, tag="pmslab")
            nc.vector.tensor_tensor(
                out=pmslab,
                in0=_ap(i512f[:, 0:1], [(0, GRP), (1, CHUNK)]),
                in1=_ap(rcol, [(SCW, GRP), (0, CHUNK)]),
                op=Alu.is_lt,
            )

            for i in range(GRP):
                b = b0 + i
                xt = xtiles[b]
                xts.append(xt)

                for ci in range(NCHUNK):
                    nc.vector.bn_stats(
                        grec[:, i, ci, :], xt[:, ci * CHUNK : (ci + 1) * CHUNK]
                    )

                pmx = pp.tile([F, CHUNK], f32, tag="pmx")
                if static_partial:
                    # timing-harness mode: same op shapes, no register APs
                    part = xt[:, (NCHUNK - 1) * CHUNK :]
                else:
                    part = xt[:, bass.ds(rvs[b], CHUNK)]
                nc.vector.tensor_mul(pmx, part, pmslab[:, i, :])
                nc.vector.bn_stats(grec[:, i, NCHUNK, :], pmx)

            # ---- record algebra, batched over the pair ([F, GRP*NREC*2]) ----
            means = grec[:, :, :, 1::3]  # [F, GRP, NREC, 2]
            m2s = grec[:, :, :, 2::3]

            s1g = sp.tile([F, GRP, NREC, 2], f32, tag="s1g")
            nc.scalar.mul(s1g, means, float(CHUNK // 2))
            mm = sp.tile([F, GRP, NREC, 2], f32, tag="mm")
            nc.vector.tensor_mul(mm, means, means)
            s2g = sp.tile([F, GRP, NREC, 2], f32, tag="s2g")
            nc.scalar.mul(s2g, mm, float(CHUNK // 2))
            nc.vector.tensor_add(s2g, s2g, m2s)

            cmask = sp.tile([F, GRP, NREC, 2], f32, tag="cmask")
            nc.vector.tensor_tensor(
                out=cmask,
                in0=_ap(i18f[:, 0:1, 0:1], [(0, GRP), (2, NREC), (1, 2)]),
                in1=_ap(kpair[:, 0:1], [(SCW, GRP), (0, NREC), (0, 2)]),
                op=Alu.is_lt,
            )
            nc.vector.tensor_mul(s1g, s1g, cmask)
            nc.vector.tensor_mul(s2g, s2g, cmask)

            s1 = sp.tile([F, GRP], f32, tag="s1")
            nc.vector.tensor_reduce(
                out=s1, in_=s1g.rearrange("p g a b -> p g (a b)"),
                axis=mybir.AxisListType.X, op=Alu.add,
            )
            s2 = sp.tile([F, GRP], f32, tag="s2")
            nc.vector.tensor_reduce(
                out=s2, in_=s2g.rearrange("p g a b -> p g (a b)"),
                axis=mybir.AxisListType.X, op=Alu.add,
            )

            # ---- finalize, batched over the pair ([F, GRP]) ----
            mean = sp.tile([F, GRP], f32, tag="mean")
            nc.vector.tensor_mul(mean, s1, inpair)
            t2 = sp.tile([F, GRP], f32, tag="t2")
            nc.vector.tensor_mul(t2, s1, mean)
            t3 = sp.tile([F, GRP], f32, tag="t3")
            nc.vector.tensor_tensor(out=t3, in0=s2, in1=t2, op=Alu.subtract)
            var = sp.tile([F, GRP], f32, tag="var")
            nc.vector.tensor_mul(var, t3, inm1pair)
            nc.vector.tensor_tensor(
                out=var, in0=var, in1=_ap(zcol, [(0, GRP)]), op=Alu.max
            )
            std = sp.tile([F, GRP], f32, tag="std")
            nc.scalar.sqrt(std, var)
            den = sp.tile([F, GRP], f32, tag="den")
            nc.vector.tensor_tensor(
                out=den, in0=std, in1=_ap(ecol, [(0, GRP)]), op=Alu.add
            )
            inv = sp.tile([F, GRP], f32, tag="inv")
            nc.vector.reciprocal(inv, den)
            nmi = sp.tile([F, GRP], f32, tag="nmi")
            nc.vector.tensor_mul(nmi, mean, inv)
            nc.vector.tensor_tensor(
                out=nmi, in0=_ap(zcol, [(0, GRP)]), in1=nmi, op=Alu.subtract
            )

            # ---- normalize + store per sample ----
            for i in range(GRP):
                b = b0 + i
                xt = xts[i]
                nc.scalar.activation(
                    out=xt, in_=xt, func=Act.Identity,
                    bias=nmi[:, i : i + 1], scale=inv[:, i : i + 1],
                )
                nc.sync.dma_start(out=out[b, :, :], in_=xt)


def _split_multi_waits(nc):
    """This walrus build allows at most one sync wait per instruction.

    Tile emits several; hoist all but one onto sequencer-only
    InstEventSemaphore instructions spliced immediately before the
    instruction on the same engine (order-preserving, so semantics are
    unchanged: waits are a conjunction).
    """
    import copy

    import bass_rust

    scratch = bass.Bass("TRN2")
    with scratch.semaphore("tmpl_sem") as s:
        tmpl = scratch.vector.wait_ge(s, 1).ins

    uid = [0]

    def make_wait(engine, wait):
        ins = copy.copy(tmpl)
        uid[0] += 1
        ins.name = f"WSPLIT-{uid[0]}"
        ins.engine = engine
        ins.sync_info = bass_rust.SyncInfo(on_wait=[wait], on_update=[])
        return ins

    spread_engines = [
        mybir.EngineType.Pool,
        mybir.EngineType.Activation,
        mybir.EngineType.PE,
        mybir.EngineType.DVE,
        mybir.EngineType.SP,
    ]

    for fn in nc.m.functions:
        for blk in fn.blocks:
            out_list = []
            changed = False
            for ins in blk.instructions:
                si = ins.sync_info
                waits = list(si.on_wait) if (si and si.on_wait) else []
                if len(waits) > 1:
                    changed = True
                    # A drain is always immediately followed by an all-engine
                    # barrier (Tile epilogue invariant in this loop-free
                    # program), so its extra waits may run on ANY engine: the
                    # barrier only completes after every engine's waits clear.
                    # Spreading them avoids a serial wait chain in the tail.
                    # For ordinary instructions the waits must stay on the
                    # same engine to order against the instruction itself.
                    is_drain = type(ins).__name__ == "InstDrain"
                    for k, w in enumerate(waits[:-1]):
                        eng = (
                            spread_engines[k % len(spread_engines)]
                            if is_drain
                            else ins.engine
                        )
                        out_list.append(make_wait(eng, w))
                    ins.sync_info = bass_rust.SyncInfo(
                        on_wait=[waits[-1]], on_update=list(si.on_update or [])
                    )
                out_list.append(ins)
            if changed:
                blk.instructions = out_list


def _hoist_head_dmas(nc, max_hoist=2):
    """Start the first input DMAs during the kernel-entry barrier.

    The first transfers have no waits (external inputs into fresh tiles),
    but Tile places them after the entry all-engine barrier, costing ~1us
    of dead DMA time.  Move up to `max_hoist` leading wait-free SP DMACopy
    instructions from the body block into `main`, after SP's preamble
    drain (so queue-base register init and quiesce still precede them)
    and before SP's barrier event-semaphore.
    """
    fn = nc.m.functions[0]
    blocks = {b.name: b for b in fn.blocks}
    main = blocks.get("main")
    if main is None or len(fn.blocks) < 2:
        return
    body = fn.blocks[1]

    hoist = []
    for ins in body.instructions:
        if len(hoist) >= max_hoist:
            break
        if (
            type(ins).__name__ == "InstDMACopy"
            and ins.engine == mybir.EngineType.SP
            and not (ins.sync_info and ins.sync_info.on_wait)
        ):
            hoist.append(ins)
    if not hoist:
        return

    insert_at = None
    for idx, ins in enumerate(main.instructions):
        if (
            type(ins).__name__ == "InstEventSemaphore"
            and ins.engine == mybir.EngineType.SP
            and str(ins.name).startswith("barrier_SP")
        ):
            insert_at = idx
            break
    if insert_at is None:
        return

    names = {h.name for h in hoist}
    body.instructions = [i for i in body.instructions if i.name not in names]
    main.instructions = (
        main.instructions[:insert_at] + hoist + main.instructions[insert_at:]
    )


def _replace_range_clear(nc):
    """This walrus rejects EVENT_SEMAPHORE_RANGE_CLEAR ("ISA wrong length").

    The clear runs after a full barrier in a loop-free program, so each
    semaphore's value there is statically known: the sum of all updates
    from instructions before the clear.  Replace the range-clear with
    explicit sem-sub-imm event-semaphore instructions restoring each sem
    in the range to zero.
    """
    import copy

    import bass_rust

    scratch = bass.Bass("TRN2")
    with scratch.semaphore("tmpl_sem") as s:
        tmpl = scratch.vector.wait_ge(s, 1).ins

    for fn in nc.m.functions:
        clears = []
        totals = {}
        names = {}
        seen_clear = False
        for blk in fn.blocks:
            for idx, ins in enumerate(blk.instructions):
                if (
                    type(ins).__name__ == "InstISA"
                    and getattr(ins, "isa_opcode", None) == 176
                ):
                    clears.append((blk, idx, ins))
                    seen_clear = True
                    continue
                si = ins.sync_info
                if not seen_clear and si and si.on_update:
                    for u in si.on_update:
                        sign = -1 if u.update_mode in ("sem-dec", "sem-sub-imm") else 1
                        totals[u.id] = totals.get(u.id, 0) + sign * u.update_value
                        names[u.id] = u.ant_name
        assert len(clears) <= 1, "multiple sem range clears unsupported"
        engines = [
            mybir.EngineType.Pool,
            mybir.EngineType.Activation,
            mybir.EngineType.PE,
            mybir.EngineType.DVE,
            mybir.EngineType.SP,
        ]
        for blk, idx, ins in clears:
            lo = ins.ant_dict["range_first"]
            hi = ins.ant_dict["range_last"]
            repl = []
            uid = 0
            for sid in range(lo, hi + 1):
                v = totals.get(sid, 0)
                assert v >= 0, f"sem {sid} negative at clear: {v}"
                if v == 0:
                    continue
                dec = copy.copy(tmpl)
                uid += 1
                dec.name = f"SEMCLR-{ins.name}-{uid}"
                # spread across engines: the clears sit between the two exit
                # barriers, so every engine is quiescent and any may clear
                dec.engine = engines[uid % len(engines)]
                dec.sync_info = bass_rust.SyncInfo(
                    on_wait=[],
                    on_update=[
                        bass_rust.SyncUpdate(
                            sync_type="semaphore",
                            id=sid,
                            ant_name=names.get(sid, f"sem{sid}"),
                            update_mode="sem-sub-imm",
                            update_value=v,
                            update_reg=None,
                        )
                    ],
                )
                repl.append(dec)
            blk.instructions = (
                blk.instructions[:idx] + repl + blk.instructions[idx + 1 :]
            )


_NC_CACHE = None


def _get_nc():
    global _NC_CACHE
    if _NC_CACHE is None:
        nc = bass.Bass("TRN2")
        x = nc.dram_tensor("x", [BPC, F, T], f32, kind="ExternalInput")
        scal = nc.dram_tensor("scal", [F, BPC, 5], f32, kind="ExternalInput")
        out = nc.dram_tensor("out", [BPC, F, T], f32, kind="ExternalOutput")
        with tile.TileContext(nc) as tc:
            build_tile_kernel(tc, x, scal, out)
        _split_multi_waits(nc)
        _replace_range_clear(nc)
        _hoist_head_dmas(nc)
        _NC_CACHE = nc
    return _NC_CACHE


def host_scalars(length):
    """Derive per-sample scalar inputs from length on host."""
    L = np.asarray(length).astype(np.int64)
    kfull = L // CHUNK
    rem = L % CHUNK
    koff = (np.minimum(kfull, NCHUNK - 1) * CHUNK).astype(np.int32)
    n = L.astype(np.float64)
    scal = np.stack(
        [
            kfull.astype(np.float64),
            rem.astype(np.float64),
            1.0 / n,
            1.0 / (n - 1.0),
        ],
        axis=1,
    ).astype(np.float32)
    # column 4: partial-chunk element offset, int32 bits viewed as f32
    scal = np.concatenate([scal, koff.view(np.float32)[:, None]], axis=1)
    return scal


TRACE = False
LAST_RESULTS = None


def kernel(x, length):
    global LAST_RESULTS
    x_np = np.asarray(x)
    length_np = np.asarray(length)
    assert x_np.shape == (B, C, F, T), x_np.shape
    x_np = np.ascontiguousarray(x_np.reshape(B, F, T).astype(np.float32, copy=False))

    scal = host_scalars(length_np)

    in_maps = []
    for c in range(NCORES):
        sl = slice(c * BPC, (c + 1) * BPC)
        in_maps.append(
            {
                "x": x_np[sl],
                "scal": np.ascontiguousarray(
                    np.broadcast_to(scal[sl][None, :, :], (F, BPC, 5))
                ),
            }
        )

    nc = _get_nc()
    res = run_bass_kernel_spmd(nc, in_maps, core_ids=list(range(NCORES)), trace=TRACE)
    LAST_RESULTS = res

    out = np.empty((B, F, T), dtype=np.float32)
    for c in range(NCORES):
        out[c * BPC : (c + 1) * BPC] = res.results[c]["out"]
    return out.reshape(B, C, F, T), length_np


# revision 44
# speedup vs baseline: 58069.6618x; 1.0030x over previous
"""CMVN kernel for Trainium2 (Bass/Tile), data-parallel over batch on 8 cores.

Problem: x [64,1,120,4096] f32, length [64] int. Per-sample mean/std over the
first length[b] frames (ddof=1), then normalize the entire padded tensor:
out = (x - mean) / (1e-10 + std). Returns (out, length).

Per-core plan (8 samples, each [120 part, 4096 free] in SBUF):
 - 8x bn_stats over 512-wide chunks -> per-chunk (mean, M2) records
 - ragged partial chunk: runtime register slice (values_load + ds) masked
   by (iota < length % 512) on GpSimd, multiplied on DVE, then bn_stats on
   the masked window forms a 9th record (zeros-as-data algebra is exact
   for plain S1/S2 sums)
 - record algebra / chunk masking / stats finalize batched per PAIR of
   samples via strided + step-0 broadcast APs (cuts tiny-op overhead)
 - normalize with one ScalarE activation(Identity, scale=1/(eps+std),
   bias=-mean*scale) pass per sample, DMA out.
Host only derives tiny per-sample scalars from `length` (chunk counts,
remainders, 1/n) - all per-element math runs on device.
"""

import numpy as np

import concourse.bass as bass
import concourse.tile as tile
from concourse import mybir
from concourse.bass_utils import run_bass_kernel_spmd

# Problem constants (hardcoded per task contract).
B, C, F, T = 64, 1, 120, 4096
NCORES = 8
BPC = B // NCORES  # samples per core
NCHUNK = 8
CHUNK = T // NCHUNK  # 512 == BN_STATS_FMAX
NREC = NCHUNK + 1  # 8 full chunks + 1 masked partial window
GRP = 2  # samples per algebra group
SCW = 5  # scal table width (kfull, rem, 1/n, 1/(n-1), koff-as-f32-bits)
EPS = 1e-10

f32 = mybir.dt.float32
i32 = mybir.dt.int32
Alu = mybir.AluOpType
Act = mybir.ActivationFunctionType


def _ap(col, dims_steps):
    """AP over `col`'s tensor starting at col's offset with explicit
    [step, count] free dims (partition dim copied from col)."""
    return bass.AP(
        tensor=col.tensor,
        offset=col.offset,
        ap=[col.ap[0]] + [[s, n] for s, n in dims_steps],
    )


def build_tile_kernel(tc, x, scal, out, reps=1, static_partial=False):
    """Emit the per-core program.

    x     [BPC, F, T] f32   input samples
    scal  [F, BPC, 5] f32   per-sample scalars (host-replicated over F):
                            kfull, rem, 1/n, 1/(n-1), bitcast-int32 element
                            offset of the partial chunk (min(kfull,7)*CHUNK)
    out   [BPC, F, T] f32   normalized output
    reps  repeat the whole pipeline (timing harness only; reps=1 for real use)
    """
    nc = tc.nc
    with (
        tc.tile_pool(name="singles", bufs=1) as singles,
        tc.tile_pool(name="xp", bufs=8) as xp,
        tc.tile_pool(name="sp", bufs=3) as sp,
        tc.tile_pool(name="pp", bufs=3) as pp,
    ):
        # First sample load leads (nothing depends on it and it's on the
        # critical DMA path); the tiny scalar table follows immediately and
        # lands long before the first pair's algebra needs it.
        xt0 = xp.tile([F, T], f32, tag="xt")
        nc.sync.dma_start(out=xt0, in_=x[0, :, :])

        sc = singles.tile([F, BPC, 5], f32)
        nc.sync.dma_start(out=sc, in_=scal[:, :, :])

        rvs = [
            nc.values_load(
                sc[0:1, b, 4:5].bitcast(i32),
                engines=(mybir.EngineType.DVE,),
                min_val=0,
                max_val=(NCHUNK - 1) * CHUNK,
                skip_runtime_bounds_check=True,
            )
            for b in range(BPC)
        ]

        zcol = singles.tile([F, 1], f32)
        nc.vector.memset(zcol, 0.0)
        ecol = singles.tile([F, 1], f32)
        nc.vector.memset(ecol, float(EPS))

        # iota constants: 0..CHUNK-1 (partial-window mask) and the record
        # chunk-index table (chunk c for records 0-7, -1 for the always-on
        # partial record), repeated per bn_stats even/odd group.
        i512 = singles.tile([F, CHUNK], i32)
        nc.gpsimd.iota(i512, pattern=[[1, CHUNK]], base=0, channel_multiplier=0)
        i512f = singles.tile([F, CHUNK], f32)
        nc.vector.tensor_copy(i512f, i512)

        i18 = singles.tile([F, NREC, 2], i32)
        nc.gpsimd.iota(i18, pattern=[[1, NREC], [0, 2]], base=0, channel_multiplier=0)
        i18f = singles.tile([F, NREC, 2], f32)
        nc.vector.tensor_copy(i18f, i18)
        nc.vector.memset(i18f[:, NCHUNK, :], -1.0)

        # body emitted `reps` times for the timing harness; reps=1 in production
        xtiles = []
        for rep, g in [(r, gg) for r in range(reps) for gg in range(BPC // GRP)]:
            if g == 0:
                # per-rep: kick off all sample loads back-to-back
                xtiles = []
                for b in range(BPC):
                    if rep == 0 and b == 0:
                        xtiles.append(xt0)
                        continue
                    xt = xp.tile([F, T], f32, tag="xt")
                    nc.sync.dma_start(out=xt, in_=x[b, :, :])
                    xtiles.append(xt)
            b0 = g * GRP
            # Pair-strided scalar APs ([F, GRP] views into sc).
            kpair = sc[:, b0 : b0 + GRP, 0]
            rcol = sc[:, b0 : b0 + GRP, 1:2]  # base for broadcast
            inpair = sc[:, b0 : b0 + GRP, 2]
            inm1pair = sc[:, b0 : b0 + GRP, 3]

            xts = []
            grec = sp.tile([F, GRP, NREC, 6], f32, tag="grec")

            # Partial-window masks for the whole pair: pm[i, t] = (t < rem_i).
            pmslab = pp.tile([F, GRP, CHUNK], f32, tag="pmslab")
            nc.vector.tensor_tensor(
                out=pmslab,
                in0=_ap(i512f[:, 0:1], [(0, GRP), (1, CHUNK)]),
                in1=_ap(rcol, [(SCW, GRP), (0, CHUNK)]),
                op=Alu.is_lt,
            )

            for i in range(GRP):
                b = b0 + i
                xt = xtiles[b]
                xts.append(xt)

                for ci in range(NCHUNK):
                    nc.vector.bn_stats(
                        grec[:, i, ci, :], xt[:, ci * CHUNK : (ci + 1) * CHUNK]
                    )

                pmx = pp.tile([F, CHUNK], f32, tag="pmx")
                if static_partial:
                    # timing-harness mode: same op shapes, no register APs
                    part = xt[:, (NCHUNK - 1) * CHUNK :]
                else:
                    part = xt[:, bass.ds(rvs[b], CHUNK)]
                nc.vector.tensor_mul(pmx, part, pmslab[:, i, :])
                nc.vector.bn_stats(grec[:, i, NCHUNK, :], pmx)

            # ---- record algebra, batched over the pair ([F, GRP*NREC*2]) ----
            means = grec[:, :, :, 1::3]  # [F, GRP, NREC, 2]
            m2s = grec[:, :, :, 2::3]

            s1g = sp.tile([F, GRP, NREC, 2], f32, tag="s1g")
            nc.scalar.mul(s1g, means, float(CHUNK // 2))
            mm = sp.tile([F, GRP, NREC, 2], f32, tag="mm")
            nc.vector.tensor_mul(mm, means, means)
            s2g = sp.tile([F, GRP, NREC, 2], f32, tag="s2g")
            nc.scalar.mul(s2g, mm, float(CHUNK // 2))
            nc.vector.tensor_add(s2g, s2g, m2s)

            cmask = sp.tile([F, GRP, NREC, 2], f32, tag="cmask")
            nc.vector.tensor_tensor(
                out=cmask,
                in0=_ap(i18f[:, 0:1, 0:1], [(0, GRP), (2, NREC), (1, 2)]),
                in1=_ap(kpair[:, 0:1], [(SCW, GRP), (0, NREC), (0, 2)]),
                op=Alu.is_lt,
            )
            nc.vector.tensor_mul(s1g, s1g, cmask)
            nc.vector.tensor_mul(s2g, s2g, cmask)

            s1 = sp.tile([F, GRP], f32, tag="s1")
            nc.vector.tensor_reduce(
                out=s1, in_=s1g.rearrange("p g a b -> p g (a b)"),
                axis=mybir.AxisListType.X, op=Alu.add,
            )
            s2 = sp.tile([F, GRP], f32, tag="s2")
            nc.vector.tensor_reduce(
                out=s2, in_=s2g.rearrange("p g a b -> p g (a b)"),
                axis=mybir.AxisListType.X, op=Alu.add,
            )

            # ---- finalize, batched over the pair ([F, GRP]) ----
            mean = sp.tile([F, GRP], f32, tag="mean")
            nc.vector.tensor_mul(mean, s1, inpair)
            t2 = sp.tile([F, GRP], f32, tag="t2")
            nc.vector.tensor_mul(t2, s1, mean)
            t3 = sp.tile([F, GRP], f32, tag="t3")
            nc.vector.tensor_tensor(out=t3, in0=s2, in1=t2, op=Alu.subtract)
            var = sp.tile([F, GRP], f32, tag="var")
            nc.vector.tensor_mul(var, t3, inm1pair)
            nc.vector.tensor_tensor(
                out=var, in0=var, in1=_ap(zcol, [(0, GRP)]), op=Alu.max
            )
            std = sp.tile([F, GRP], f32, tag="std")
            nc.scalar.sqrt(std, var)
            den = sp.tile([F, GRP], f32, tag="den")
            nc.vector.tensor_tensor(
                out=den, in0=std, in1=_ap(ecol, [(0, GRP)]), op=Alu.add
            )
            inv = sp.tile([F, GRP], f32, tag="inv")
            nc.vector.reciprocal(inv, den)
            nmi = sp.tile([F, GRP], f32, tag="nmi")
            nc.vector.tensor_mul(nmi, mean, inv)
            nc.vector.tensor_tensor(
                out=nmi, in0=_ap(zcol, [(0, GRP)]), in1=nmi, op=Alu.subtract
            )

            # ---- normalize + store per sample ----
            for i in range(GRP):
                b = b0 + i
                xt = xts[i]
                nc.scalar.activation(
                    out=xt, in_=xt, func=Act.Identity,
                    bias=nmi[:, i : i + 1], scale=inv[:, i : i + 1],
                )
                nc.sync.dma_start(out=out[b, :, :], in_=xt)


def _split_multi_waits(nc):
    """This walrus build allows at most one sync wait per instruction.

    Tile emits several; hoist all but one onto sequencer-only
    InstEventSemaphore instructions spliced immediately before the
    instruction on the same engine (order-preserving, so semantics are
    unchanged: waits are a conjunction).
    """
    import copy

    import bass_rust

    scratch = bass.Bass("TRN2")
    with scratch.semaphore("tmpl_sem") as s:
        tmpl = scratch.vector.wait_ge(s, 1).ins

    uid = [0]

    def make_wait(engine, wait):
        ins = copy.copy(tmpl)
        uid[0] += 1
        ins.name = f"WSPLIT-{uid[0]}"
        ins.engine = engine
        ins.sync_info = bass_rust.SyncInfo(on_wait=[wait], on_update=[])
        return ins

    spread_engines = [
        mybir.EngineType.Pool,
        mybir.EngineType.Activation,
        mybir.EngineType.PE,
        mybir.EngineType.DVE,
        mybir.EngineType.SP,
    ]

    for fn in nc.m.functions:
        for blk in fn.blocks:
            out_list = []
            changed = False
            for ins in blk.instructions:
                si = ins.sync_info
                waits = list(si.on_wait) if (si and si.on_wait) else []
                if len(waits) > 1:
                    changed = True
                    # A drain is always immediately followed by an all-engine
                    # barrier (Tile epilogue invariant in this loop-free
                    # program), so its extra waits may run on ANY engine: the
                    # barrier only completes after every engine's waits clear.
                    # Spreading them avoids a serial wait chain in the tail.
                    # For ordinary instructions the waits must stay on the
                    # same engine to order against the instruction itself.
                    is_drain = type(ins).__name__ == "InstDrain"
                    for k, w in enumerate(waits[:-1]):
                        eng = (
                            spread_engines[k % len(spread_engines)]
                            if is_drain
                            else ins.engine
                        )
                        out_list.append(make_wait(eng, w))
                    ins.sync_info = bass_rust.SyncInfo(
                        on_wait=[waits[-1]], on_update=list(si.on_update or [])
                    )
                out_list.append(ins)
            if changed:
                blk.instructions = out_list


def _hoist_head_dmas(nc, max_hoist=2):
    """Start the first input DMAs during the kernel-entry barrier.

    The first transfers have no waits (external inputs into fresh tiles),
    but Tile places them after the entry all-engine barrier, costing ~1us
    of dead DMA time.  Move up to `max_hoist` leading wait-free SP DMACopy
    instructions from the body block into `main`, after SP's preamble
    drain (so queue-base register init and quiesce still precede them)
    and before SP's barrier event-semaphore.
    """
    fn = nc.m.functions[0]
    blocks = {b.name: b for b in fn.blocks}
    main = blocks.get("main")
    if main is None or len(fn.blocks) < 2:
        return
    body = fn.blocks[1]

    hoist = []
    for ins in body.instructions:
        if len(hoist) >= max_hoist:
            break
        if (
            type(ins).__name__ == "InstDMACopy"
            and ins.engine == mybir.EngineType.SP
            and not (ins.sync_info and ins.sync_info.on_wait)
        ):
            hoist.append(ins)
    if not hoist:
        return

    # Insert before SP's first preamble instruction: those RegisterMoves
    # only init zero/branch-condition scratch regs, which plain DMACopy
    # never reads, so the descriptor generation can lead the stream.
    insert_at = None
    for idx, ins in enumerate(main.instructions):
        if ins.engine == mybir.EngineType.SP:
            insert_at = idx
            break
    if insert_at is None:
        return

    names = {h.name for h in hoist}
    body.instructions = [i for i in body.instructions if i.name not in names]
    main.instructions = (
        main.instructions[:insert_at] + hoist + main.instructions[insert_at:]
    )


def _replace_range_clear(nc):
    """This walrus rejects EVENT_SEMAPHORE_RANGE_CLEAR ("ISA wrong length").

    The clear runs after a full barrier in a loop-free program, so each
    semaphore's value there is statically known: the sum of all updates
    from instructions before the clear.  Replace the range-clear with
    explicit sem-sub-imm event-semaphore instructions restoring each sem
    in the range to zero.
    """
    import copy

    import bass_rust

    scratch = bass.Bass("TRN2")
    with scratch.semaphore("tmpl_sem") as s:
        tmpl = scratch.vector.wait_ge(s, 1).ins

    for fn in nc.m.functions:
        clears = []
        totals = {}
        names = {}
        seen_clear = False
        for blk in fn.blocks:
            for idx, ins in enumerate(blk.instructions):
                if (
                    type(ins).__name__ == "InstISA"
                    and getattr(ins, "isa_opcode", None) == 176
                ):
                    clears.append((blk, idx, ins))
                    seen_clear = True
                    continue
                si = ins.sync_info
                if not seen_clear and si and si.on_update:
                    for u in si.on_update:
                        sign = -1 if u.update_mode in ("sem-dec", "sem-sub-imm") else 1
                        totals[u.id] = totals.get(u.id, 0) + sign * u.update_value
                        names[u.id] = u.ant_name
        assert len(clears) <= 1, "multiple sem range clears unsupported"
        engines = [
            mybir.EngineType.Pool,
            mybir.EngineType.Activation,
            mybir.EngineType.PE,
            mybir.EngineType.DVE,
            mybir.EngineType.SP,
        ]
        for blk, idx, ins in clears:
            lo = ins.ant_dict["range_first"]
            hi = ins.ant_dict["range_last"]
            repl = []
            uid = 0
            for sid in range(lo, hi + 1):
                v = totals.get(sid, 0)
                assert v >= 0, f"sem {sid} negative at clear: {v}"
                if v == 0:
                    continue
                dec = copy.copy(tmpl)
                uid += 1
                dec.name = f"SEMCLR-{ins.name}-{uid}"
                # spread across engines: the clears sit between the two exit
                # barriers, so every engine is quiescent and any may clear
                dec.engine = engines[uid % len(engines)]
                dec.sync_info = bass_rust.SyncInfo(
                    on_wait=[],
                    on_update=[
                        bass_rust.SyncUpdate(
                            sync_type="semaphore",
                            id=sid,
                            ant_name=names.get(sid, f"sem{sid}"),
                            update_mode="sem-sub-imm",
                            update_value=v,
                            update_reg=None,
                        )
                    ],
                )
                repl.append(dec)
            blk.instructions = (
                blk.instructions[:idx] + repl + blk.instructions[idx + 1 :]
            )


_NC_CACHE = None


def _get_nc():
    global _NC_CACHE
    if _NC_CACHE is None:
        nc = bass.Bass("TRN2")
        x = nc.dram_tensor("x", [BPC, F, T], f32, kind="ExternalInput")
        scal = nc.dram_tensor("scal", [F, BPC, 5], f32, kind="ExternalInput")
        out = nc.dram_tensor("out", [BPC, F, T], f32, kind="ExternalOutput")
        with tile.TileContext(nc) as tc:
            build_tile_kernel(tc, x, scal, out)
        _split_multi_waits(nc)
        _replace_range_clear(nc)
        _hoist_head_dmas(nc)
        _NC_CACHE = nc
    return _NC_CACHE


def host_scalars(length):
    """Derive per-sample scalar inputs from length on host."""
    L = np.asarray(length).astype(np.int64)
    kfull = L // CHUNK
    rem = L % CHUNK
    koff = (np.minimum(kfull, NCHUNK - 1) * CHUNK).astype(np.int32)
    n = L.astype(np.float64)
    scal = np.stack(
        [
            kfull.astype(np.float64),
            rem.astype(np.float64),
            1.0 / n,
            1.0 / (n - 1.0),
        ],
        axis=1,
    ).astype(np.float32)
    # column 4: partial-chunk element offset, int32 bits viewed as f32
    scal = np.concatenate([scal, koff.view(np.float32)[:, None]], axis=1)
    return scal


TRACE = False
LAST_RESULTS = None


def kernel(x, length):
    global LAST_RESULTS
    x_np = np.asarray(x)
    length_np = np.asarray(length)
    assert x_np.shape == (B, C, F, T), x_np.shape
    x_np = np.ascontiguousarray(x_np.reshape(B, F, T).astype(np.float32, copy=False))

    scal = host_scalars(length_np)

    in_maps = []
    for c in range(NCORES):
        sl = slice(c * BPC, (c + 1) * BPC)
        in_maps.append(
            {
                "x": x_np[sl],
                "scal": np.ascontiguousarray(
                    np.broadcast_to(scal[sl][None, :, :], (F, BPC, 5))
                ),
            }
        )

    nc = _get_nc()
    res = run_bass_kernel_spmd(nc, in_maps, core_ids=list(range(NCORES)), trace=TRACE)
    LAST_RESULTS = res

    out = np.empty((B, F, T), dtype=np.float32)
    for c in range(NCORES):
        out[c * BPC : (c + 1) * BPC] = res.results[c]["out"]
    return out.reshape(B, C, F, T), length_np
